# Initial kernel scaffold
#
"""Trainium2 Bass kernel for nn_ExtSummarizer (B=512, S=100, H=768).

Math (per batch b, mask==1, true_dim==S):
  off[i] = s_i . u + b,  u = W_rel d + W_cont^T,  d = mean_i s_i
  q = sigmoid(s W_sim s^T + off[:,None])
  sv[j] = sum_i q[i,j];  solve (I - lam*q*diag(1/sv)) x = y,  y = 1/S
  score = (1-lam) x

Device algorithm (transposed formulation):
  - compute simT[j,i] = (s W_sim s^T)^T via matmul(lhsT=sentT, rhs=YT);
    off[i] enters along the free axis via a K=1 accumulating matmul
    (lhsT=ones-row, rhs=off_row), off_row from matmul(lhsT=u [768,1]).
  - one ACT sigmoid produces qT and, via accum_out, sv[j] (per-partition).
  - operator N = lam * q * diag(1/sv):  NT = lam * diag(1/sv) * qT is a
    per-partition scale of qT;  N = PE-transpose(NT).
  - x = sum_{k<2^J} N^k y,  y = (1-lam)/S * ones  (scale folded into z0);
    z-doubling with both N and NT kept so squaring needs no transposes:
      [N^2 | N z] = matmul(lhsT=NT, rhs=[N | z]);   (N^2)^T = matmul(lhsT=N, rhs=NT)
    solve runs level-by-level over all batches, packed 5 batches per PSUM
    bank, drained with one DVE + one ACT op per pack.

Sharding: pure data parallel, 64 batches per core, 8 cores.
All matmuls fp16 operands with fp32 PSUM accumulate. d_rep comes from the
host (fp32 math) as an input.
"""

import numpy as np

B, S, H = 512, 100, 768
NCORES = 8
BC = B // NCORES          # 64 batches per core
ROWS = BC * S             # 6400 rows per core
LAMB = 0.8
NGRP = 2                  # row-groups per core
GB = BC // NGRP           # 32 batches per group
GROWS = GB * S            # 3200 rows per group
HC = H // 128             # 6 k-chunks
NT = 400                  # mm1 moving-dim tile (8 * 400 = 3200)
NNT = GROWS // NT
J = 5                     # Neumann doubling: covers k < 2^J = 64
PACK = 5                  # batches per PSUM bank in the solve levels
SZ = S + 1                # N tile row length (N | z)
Z0 = (1.0 - LAMB) / S     # z start value (y scaled by the final (1-lam))

_CACHE = {}


def _get_nc(loop_n=1):
    """Build (and cache) the per-core Bass program.

    loop_n > 1 wraps the whole body in a device-side For_i loop — used only
    by the timing harness to amortize the host dispatch overhead."""
    key = ("nc", loop_n)
    if key in _CACHE:
        return _CACHE[key]

    import contextlib

    import concourse.bass as bass
    import concourse.mybir as mybir
    import concourse.tile as tile
    from concourse import bacc
    from concourse.bass import ts

    fp16 = mybir.dt.float16
    fp32 = mybir.dt.float32
    AF = mybir.ActivationFunctionType
    OP = mybir.AluOpType
    X = mybir.AxisListType.X

    nc = bacc.Bacc(trn_type="TRN2", target_bir_lowering=False, debug=False)

    sent16 = nc.dram_tensor("sent16", [128, HC, ROWS], fp16, kind="ExternalInput")
    wsim16 = nc.dram_tensor("wsim16", [H, H], fp16, kind="ExternalInput")
    wrelT16 = nc.dram_tensor("wrelT16", [H, H], fp16, kind="ExternalInput")
    wcont32 = nc.dram_tensor("wcont32", [H], fp32, kind="ExternalInput")
    d16h = nc.dram_tensor("d16h", [H, BC], fp16, kind="ExternalInput")
    onesr16 = nc.dram_tensor("onesr16", [1, S], fp16, kind="ExternalInput")
    eye16 = nc.dram_tensor("eye16", [S, S], fp16, kind="ExternalInput")
    bvec32 = nc.dram_tensor("bvec32", [S, 1], fp32, kind="ExternalInput")
    out32 = nc.dram_tensor("out32", [BC, S], fp32, kind="ExternalOutput")

    with tile.TileContext(nc) as tc:
        loop_cm = tc.For_i(0, loop_n, 1) if loop_n > 1 else contextlib.nullcontext()
        with (
            loop_cm,
            tc.tile_pool(name="const", bufs=1) as const,
            tc.tile_pool(name="sentT_p", bufs=2) as sentT_p,
            tc.tile_pool(name="yt_p", bufs=2) as yt_p,
            tc.tile_pool(name="grp_p", bufs=2) as grp_p,
            tc.tile_pool(name="solve_p", bufs=2) as solve_p,
            tc.tile_pool(name="small", bufs=6) as small,
            tc.tile_pool(name="psum", bufs=6, space="PSUM") as psum,
            tc.tile_pool(name="psmm", bufs=2, space="PSUM") as psmm,
        ):
            wsim_sb = const.tile([128, HC, H], fp16)
            nc.sync.dma_start(
                wsim_sb[:], wsim16.ap().rearrange("(c p) n -> p c n", p=128)
            )
            wrelT_sb = const.tile([128, HC, H], fp16)
            nc.sync.dma_start(
                wrelT_sb[:], wrelT16.ap().rearrange("(c p) n -> p c n", p=128)
            )
            wcont_sb = const.tile([128, HC], fp32)
            nc.sync.dma_start(
                wcont_sb[:], wcont32.ap().rearrange("(c p) -> p c", p=128)
            )
            d16_sb = const.tile([128, HC, BC], fp16)
            nc.sync.dma_start(
                d16_sb[:], d16h.ap().rearrange("(c p) b -> p c b", p=128)
            )
            onesr_sb = const.tile([1, S], fp16)
            nc.sync.dma_start(onesr_sb[:], onesr16.ap())
            eye_sb = const.tile([S, S], fp16)
            nc.sync.dma_start(eye_sb[:], eye16.ap())
            bvec_sb = const.tile([S, 1], fp32)
            nc.sync.dma_start(bvec_sb[:], bvec32.ap())

            # --- sentT loads (host pre-transposed): sentT[p, c, r]
            sentT_g = []
            for g in range(NGRP):
                sentT = sentT_p.tile(
                    [128, HC, GROWS], fp16, tag="sentT", name=f"sentT{g}"
                )
                sentT_g.append(sentT)
                nc.sync.dma_start(
                    out=sentT[:],
                    in_=sent16.ap()[:, :, g * GROWS : (g + 1) * GROWS],
                )

            for g in range(NGRP):
                sentT = sentT_g[g]

                # --- u = W_rel d + W_cont^T  (chunked like sentT)
                u16 = grp_p.tile([128, HC, GB], fp16, tag="u16")
                for m in range(HC):
                    psu = psmm.tile([128, 512], fp32, tag="mm", name=f"psu{g}{m}")
                    for c in range(HC):
                        nc.tensor.matmul(
                            psu[:, :GB],
                            wrelT_sb[:, c, m * 128 : (m + 1) * 128],
                            d16_sb[:, c, g * GB : (g + 1) * GB],
                            start=(c == 0),
                            stop=(c == HC - 1),
                        )
                    nc.vector.tensor_scalar(
                        out=u16[:, m, :],
                        in0=psu[:, :GB],
                        scalar1=wcont_sb[:, m : m + 1],
                        scalar2=None,
                        op0=OP.add,
                    )

                # --- mm1: YT[p, m, r] = (sent @ W_sim)^T, chunked layout
                yt = yt_p.tile([128, HC, GROWS], fp16, tag="yt")
                for m in range(HC):
                    for n in range(NNT):
                        psy = psmm.tile(
                            [128, 512], fp32, tag="mm", name=f"psy{g}{m}{n}"
                        )
                        for c in range(HC):
                            nc.tensor.matmul(
                                psy[:, :NT],
                                wsim_sb[:, c, m * 128 : (m + 1) * 128],
                                sentT[:, c, ts(n, NT)],
                                start=(c == 0),
                                stop=(c == HC - 1),
                            )
                        if (m * NNT + n) % 2 == 0:
                            nc.scalar.copy(yt[:, m, ts(n, NT)], psy[:, :NT])
                        else:
                            nc.vector.tensor_copy(yt[:, m, ts(n, NT)], psy[:, :NT])

                # --- phase B: per batch simT -> qT, sv -> NT, N, z0
                N_cur = solve_p.tile([S, GB * SZ], fp16, tag="Nall", name=f"N0g{g}")
                NT_cur = solve_p.tile([S, GB * S], fp16, tag="NTall", name=f"NT0g{g}")
                for bl in range(GB):
                    sl = slice(bl * S, (bl + 1) * S)
                    # simT[j, i] accumulated over 6 k-chunks, then off along free
                    ps_s = psum.tile([S, S], fp32, tag="bank", name=f"sim{g}{bl}")
                    for c in range(HC):
                        nc.tensor.matmul(
                            ps_s[:],
                            sentT[:, c, sl],
                            yt[:, c, sl],
                            start=(c == 0),
                            stop=False,
                        )
                    psm = psum.tile([128, 512], fp32, tag="bank", name=f"msc{g}{bl}")
                    offr_ps = psm[:1, 0:S]
                    Nt_ps = psm[:S, 128:178].bitcast(fp16)  # [S,100] fp16, disjoint from offr
                    for c in range(HC):
                        nc.tensor.matmul(
                            offr_ps,
                            u16[:, c, bl : bl + 1],
                            sentT[:, c, sl],
                            start=(c == 0),
                            stop=(c == HC - 1),
                        )
                    offr_sb = small.tile([1, S], fp16, tag="offr")
                    nc.vector.tensor_copy(offr_sb[:], offr_ps)
                    nc.tensor.matmul(
                        ps_s[:], onesr_sb[:], offr_sb[:], start=False, stop=True
                    )
                    qT_sb = small.tile([S, S], fp16, tag="qT")
                    sv_sb = small.tile([S, 1], fp32, tag="sv")
                    nc.scalar.activation(
                        qT_sb[:],
                        ps_s[:],
                        AF.Sigmoid,
                        bias=bvec_sb[:, 0:1],
                        scale=1.0,
                    )
                    nc.vector.reduce_sum(out=sv_sb[:], in_=qT_sb[:], axis=X)
                    r_sb = small.tile([S, 1], fp32, tag="r")
                    nc.vector.reciprocal(r_sb[:], sv_sb[:])
                    nc.vector.tensor_scalar(
                        out=NT_cur[:, sl],
                        in0=qT_sb[:],
                        scalar1=r_sb[:, 0:1],
                        scalar2=LAMB,
                        op0=OP.mult,
                        op1=OP.mult,
                    )
                    nc.tensor.transpose(Nt_ps, NT_cur[:, sl], eye_sb[:])
                    nc.scalar.copy(N_cur[:, bl * SZ : bl * SZ + S], Nt_ps)
                    nc.vector.memset(N_cur[:, bl * SZ + S : bl * SZ + SZ], Z0)

                # --- phase C: Neumann doubling, level-ordered, packed drains
                packs = []
                p0 = 0
                while p0 < GB:
                    packs.append((p0, min(PACK, GB - p0)))
                    p0 += PACK
                for j in range(J - 1):
                    N_nxt = solve_p.tile(
                        [S, GB * SZ], fp16, tag="Nall", name=f"N{j + 1}g{g}"
                    )
                    NT_nxt = solve_p.tile(
                        [S, GB * S], fp16, tag="NTall", name=f"NT{j + 1}g{g}"
                    )
                    for p0, np_ in packs:
                        sq = psum.tile(
                            [S, np_ * SZ], fp32, tag="bank", name=f"sq{g}{j}{p0}"
                        )
                        sqT = psum.tile(
                            [S, np_ * S], fp32, tag="bank", name=f"sqT{g}{j}{p0}"
                        )
                        for i in range(np_):
                            b = p0 + i
                            nc.tensor.matmul(
                                sq[:, i * SZ : (i + 1) * SZ],
                                NT_cur[:, b * S : (b + 1) * S],
                                N_cur[:, b * SZ : (b + 1) * SZ],
                                start=True,
                                stop=True,
                            )
                            nc.tensor.matmul(
                                sqT[:, i * S : (i + 1) * S],
                                N_cur[:, b * SZ : b * SZ + S],
                                NT_cur[:, b * S : (b + 1) * S],
                                start=True,
                                stop=True,
                            )
                        sq3 = sq.rearrange("p (n w) -> p n w", w=SZ)
                        dst3 = N_nxt[:, p0 * SZ : (p0 + np_) * SZ].rearrange(
                            "p (n w) -> p n w", w=SZ
                        )
                        cur_z = N_cur[:, p0 * SZ : (p0 + np_) * SZ].rearrange(
                            "p (n w) -> p n w", w=SZ
                        )[:, :, S:SZ]
                        nc.vector.tensor_copy(dst3[:, :, 0:S], sq3[:, :, 0:S])
                        nc.vector.tensor_tensor(
                            out=dst3[:, :, S:SZ],
                            in0=sq3[:, :, S:SZ],
                            in1=cur_z,
                            op=OP.add,
                        )
                        nc.scalar.copy(NT_nxt[:, p0 * S : (p0 + np_) * S], sqT[:])
                    N_cur, NT_cur = N_nxt, NT_nxt

                # --- final: x = z + N z   (z0 already carries the 0.2/S scale)
                fz = psum.tile([S, GB], fp32, tag="bank", name=f"fz{g}")
                for bl in range(GB):
                    nc.tensor.matmul(
                        fz[:, bl : bl + 1],
                        NT_cur[:, bl * S : (bl + 1) * S],
                        N_cur[:, bl * SZ + S : (bl + 1) * SZ],
                        start=True,
                        stop=True,
                    )
                xg = grp_p.tile([S, GB], fp32, tag="xg")
                zcols = N_cur.rearrange("p (n w) -> p n w", w=SZ)[:, :, S:SZ]
                nc.vector.tensor_tensor(
                    out=xg[:].rearrange("p (n w) -> p n w", w=1),
                    in0=fz[:].rearrange("p (n w) -> p n w", w=1),
                    in1=zcols,
                    op=OP.add,
                )

                nc.sync.dma_start(
                    out=out32.ap()[g * GB : (g + 1) * GB, :].rearrange("b s -> s b"),
                    in_=xg[:],
                )

    nc.compile()
    _CACHE[key] = nc
    return nc


def _prep(inputs):
    sent = np.ascontiguousarray(np.asarray(inputs["sent_vec"], dtype=np.float32))
    s16 = sent.reshape(NCORES, ROWS, HC, 128).astype(np.float16)
    sent16 = np.ascontiguousarray(s16.transpose(0, 3, 2, 1))  # [NC,128,HC,ROWS]
    d32 = sent.reshape(NCORES, BC, S, H).mean(axis=2)        # [NC, BC, H] fp32
    d16h = d32.transpose(0, 2, 1).astype(np.float16)         # [NC, H, BC]
    wsim16 = np.ascontiguousarray(
        np.asarray(inputs["W_sim"], dtype=np.float32)
    ).astype(np.float16)
    wrelT16 = np.ascontiguousarray(
        np.asarray(inputs["W_rel"], dtype=np.float32).T
    ).astype(np.float16)
    wcont = np.asarray(inputs["W_cont"], dtype=np.float32).reshape(H)
    bval = float(np.asarray(inputs["b_matrix"]).reshape(-1)[0])
    onesr = np.ones((1, S), np.float16)
    eye = np.eye(S, dtype=np.float16)
    bvec = np.full((S, 1), bval, np.float32)
    return [
        {
            "sent16": np.ascontiguousarray(sent16[i]),
            "wsim16": wsim16,
            "wrelT16": wrelT16,
            "wcont32": wcont,
            "d16h": np.ascontiguousarray(d16h[i]),
            "onesr16": onesr,
            "eye16": eye,
            "bvec32": bvec,
        }
        for i in range(NCORES)
    ]


def _patch_ldw_opt():
    import os

    if os.environ.get("KERNEL_LDW_OPT", "0") != "1":
        return
    import concourse.bass_utils as bu

    if getattr(bu, "_ldw_patched", False):
        return
    orig = bu.run_command

    def run2(argv, **kw):
        argv = [
            "--enable-ldw-opt=true" if a == "--enable-ldw-opt=false" else a
            for a in argv
        ]
        return orig(argv, **kw)

    bu.run_command = run2
    bu._ldw_patched = True


def _run(in_maps, trace=False, **kw):
    from concourse.bass_utils import run_bass_kernel_spmd

    _patch_ldw_opt()
    nc = _get_nc()
    return run_bass_kernel_spmd(nc, in_maps, list(range(NCORES)), trace=trace, **kw)


def kernel(**inputs):
    in_maps = _prep(inputs)
    res = _run(in_maps)
    out = np.concatenate([r["out32"] for r in res.results], axis=0)
    return np.ascontiguousarray(out, dtype=np.float32)


if __name__ == "__main__":
    _get_nc()
    print("build ok")



# revision 4
# speedup vs baseline: 1.7296x; 1.7296x over previous
"""Trainium2 Bass kernel for nn_ExtSummarizer (B=512, S=100, H=768).

Math (per batch b, mask==1, true_dim==S):
  off[i] = s_i . v,  v = W_rel d + W_cont^T,  d = mean_i s_i   (host, fp32)
  q = sigmoid(s W_sim s^T + off[:,None] + b)
  sv[j] = sum_i q[i,j];  solve (I - lam*q*diag(1/sv)) x = y,  y = 1/S
  score = (1-lam) x

Device algorithm (transposed formulation, fp8 e4m3 matmuls):
  - mm1: yt[h',r] = sum_h (WS*W_sim)[h,h'] s[r,h] via fp8 DoubleRow
    (K=256/instr), fp32 PSUM, drained to fp8 yt (ACT/DVE alternating).
  - per 4-batch PSUM block: simT[j,i] fp8 accumulated over K=768 (128-col
    padded lhsT for FWL); off (host fp32, *WS, fp16) enters via a K=1 fp16
    matmul on a ones row.  One blocked ACT sigmoid (scale=1/WS, bias=b)
    -> qT fp16; blocked DVE reduce -> sv; reciprocal; NT = lam*qT/sv;
    N = PE-transpose(NT), blocked drain.
  - solve x = sum_{k<24} N^k z, z=(1-lam)/S: 2 full doubling levels
    (N<-N^2, z<-z+Nz, both orientations), level 2 computing only z3 and
    (N^8)^T, then w1=N^8 z3, w2=N^8 w1, x = z3+w1+w2.  fp16, fp32 PSUM.
  - emission interleaves solve(g) with mm1/phase2(g+1) to keep the
    in-order engines fed; 4 groups of 16 batches pipeline.

Sharding: pure data parallel, 64 batches per core, 8 cores.
"""

import numpy as np
import ml_dtypes

B, S, H = 512, 100, 768
NCORES = 8
BC = B // NCORES          # 64 batches per core
ROWS = BC * S             # 6400 rows per core
LAMB = 0.8
NGRP = 4                  # row-groups per core
GB = BC // NGRP           # 16 batches per group
GROWS = GB * S            # 1600 rows per group
HC = H // 128             # 6 k-chunks
HP = HC // 2              # 3 DoubleRow k-pairs
NT = 400                  # mm1 moving-dim tile
NNT = GROWS // NT         # 4
SZ = S + 1                # N tile row length (N | z)
PACK = 4                  # batches per PSUM bank in the solve levels
NLEV = 2                  # full doubling levels
NMV = 2                   # matvec rounds (coverage: k < 2^(NLEV+1)*(NMV+1))
WS = 16.0                 # fp8 scale on W_sim / off
Z0 = (1.0 - LAMB) / S
PADC = 128                # lhsT column pad for FWL
SPAD = GROWS + PADC
E4NP = ml_dtypes.float8_e4m3

MM1_DR = True             # mm1 via DoubleRow (else plain fp8)
SIMT_DR = False           # simT via DoubleRow (else plain fp8)

_CACHE = {}


def _get_nc(loop_n=1):
    key = ("nc", loop_n)
    if key in _CACHE:
        return _CACHE[key]

    import contextlib

    import concourse.mybir as mybir
    import concourse.tile as tile
    from concourse import bacc
    from concourse.bass import ts

    fp8 = mybir.dt.float8e4
    fp16 = mybir.dt.float16
    fp32 = mybir.dt.float32
    AF = mybir.ActivationFunctionType
    OP = mybir.AluOpType
    X = mybir.AxisListType.X
    PM = mybir.MatmulPerfMode.DoubleRow

    nc = bacc.Bacc(trn_type="TRN2", target_bir_lowering=False, debug=False)

    sent8 = nc.dram_tensor("sent8", [128, HC, ROWS], fp8, kind="ExternalInput")
    wsim8 = nc.dram_tensor("wsim8", [128, HC, H], fp8, kind="ExternalInput")
    off16h = nc.dram_tensor("off16h", [1, ROWS], fp16, kind="ExternalInput")
    onesr16 = nc.dram_tensor("onesr16", [1, PADC], fp16, kind="ExternalInput")
    eye16 = nc.dram_tensor("eye16", [S, S], fp16, kind="ExternalInput")
    bvec32 = nc.dram_tensor("bvec32", [S, 1], fp32, kind="ExternalInput")
    out32 = nc.dram_tensor("out32", [BC, S], fp32, kind="ExternalOutput")

    NPADW = GB * S + PADC
    NPADZ = GB * SZ + PADC
    NBLK = GB // 4

    with tile.TileContext(nc) as tc:
        loop_cm = tc.For_i(0, loop_n, 1) if loop_n > 1 else contextlib.nullcontext()
        with (
            loop_cm,
            tc.tile_pool(name="const", bufs=1) as const,
            tc.tile_pool(name="sentT_p", bufs=2) as sentT_p,
            tc.tile_pool(name="yt_p", bufs=2) as yt_p,
            tc.tile_pool(name="grp_p", bufs=2) as grp_p,
            tc.tile_pool(name="solve_p", bufs=3) as solve_p,
            tc.tile_pool(name="small", bufs=4) as small,
            tc.tile_pool(name="psmm", bufs=2, space="PSUM") as psmm,
            tc.tile_pool(name="psb", bufs=2, space="PSUM") as psb_p,
            tc.tile_pool(name="psb2", bufs=2, space="PSUM") as psb2_p,
            tc.tile_pool(name="pssv", bufs=2, space="PSUM") as pssv_p,
        ):
            wsim_sb = const.tile([128, HC, H], fp8)
            nc.sync.dma_start(wsim_sb[:], wsim8.ap())
            off_sb = const.tile([1, ROWS], fp16)
            nc.sync.dma_start(off_sb[:], off16h.ap())
            onesr_sb = const.tile([1, PADC], fp16)
            nc.sync.dma_start(onesr_sb[:], onesr16.ap())
            eye_sb = const.tile([S, S], fp16)
            nc.sync.dma_start(eye_sb[:], eye16.ap())
            bvec_sb = const.tile([S, 1], fp32)
            nc.sync.dma_start(bvec_sb[:], bvec32.ap())

            st = {}          # per-group live tiles
            par = [0]        # drain engine parity

            def alt_copy(dst, src):
                if par[0] % 2 == 0:
                    nc.scalar.copy(dst, src)
                else:
                    nc.vector.tensor_copy(dst, src)
                par[0] += 1

            def load_thunk(g):
                def t():
                    sentT = sentT_p.tile([128, HC, SPAD], fp8, tag="sentT",
                                         name=f"sentT{g}")
                    st["sentT", g] = sentT
                    nc.gpsimd.memset(sentT[:, :, GROWS:SPAD], 0.0)
                    for n in range(NNT):
                        nc.sync.dma_start(
                            out=sentT[:, :, ts(n, NT)],
                            in_=sent8.ap()[
                                :, :, g * GROWS + n * NT : g * GROWS + (n + 1) * NT
                            ],
                        )
                return [t]

            def mm1_thunks(g):
                def start():
                    yt = yt_p.tile([128, HC, SPAD], fp8, tag="yt",
                                   name=f"yt{g}")
                    st["yt", g] = yt
                    nc.gpsimd.memset(yt[:, :, GROWS:SPAD], 0.0)
                out = [start]

                def tile_t(n, m):
                    def t():
                        sentT = st["sentT", g]
                        yt = st["yt", g]
                        psy = psmm.tile([128, 512], fp32, tag="mm",
                                        name=f"psy{g}_{n}_{m}")
                        if MM1_DR:
                            for tt in range(HP):
                                nc.tensor.matmul(
                                    psy[:, :NT],
                                    wsim_sb[:, 2 * tt : 2 * tt + 2,
                                            m * 128 : (m + 1) * 128],
                                    sentT[:, 2 * tt : 2 * tt + 2, ts(n, NT)],
                                    start=(tt == 0),
                                    stop=(tt == HP - 1),
                                    perf_mode=PM,
                                )
                        else:
                            for c in range(HC):
                                nc.tensor.matmul(
                                    psy[:, :NT],
                                    wsim_sb[:, c, m * 128 : (m + 1) * 128],
                                    sentT[:, c, ts(n, NT)],
                                    start=(c == 0),
                                    stop=(c == HC - 1),
                                )
                        alt_copy(yt[:, m, ts(n, NT)], psy[:, :NT])
                    return t

                for n in range(NNT):
                    for m in range(HC):
                        out.append(tile_t(n, m))
                return out

            def ph2_thunks(g):
                def start():
                    N_cur = solve_p.tile([S, NPADZ], fp16, tag="Nall",
                                         name=f"N0g{g}")
                    NT_cur = solve_p.tile([S, NPADW], fp16, tag="NTall",
                                          name=f"NT0g{g}")
                    st["N", g] = N_cur
                    st["NT", g] = NT_cur
                    nc.gpsimd.memset(N_cur[:, GB * SZ : NPADZ], 0.0)
                    nc.gpsimd.memset(NT_cur[:, GB * S : NPADW], 0.0)
                    nc.vector.memset(
                        N_cur[:, 0 : GB * SZ].rearrange(
                            "p (n w) -> p n w", w=SZ
                        )[:, :, S:SZ],
                        Z0,
                    )
                    st["svg", g] = grp_p.tile([S, GB], fp32, tag="svg",
                                              name=f"svg{g}")
                    st["rg", g] = grp_p.tile([S, GB], fp32, tag="rg",
                                             name=f"rg{g}")
                out = [start]

                def blk_t(blk):
                    def t():
                        sentT = st["sentT", g]
                        yt = st["yt", g]
                        N_cur, NT_cur = st["N", g], st["NT", g]
                        svg, rg = st["svg", g], st["rg", g]
                        psb = psb_p.tile([128, 512], fp32, tag="sim",
                                         name=f"sim{g}_{blk}")
                        for q in range(4):
                            bl = blk * 4 + q
                            r0 = bl * S
                            dst = psb[:, q * 128 : q * 128 + S]
                            if SIMT_DR:
                                for tt in range(HP):
                                    nc.tensor.matmul(
                                        dst,
                                        sentT[:, 2 * tt : 2 * tt + 2,
                                              r0 : r0 + PADC],
                                        yt[:, 2 * tt : 2 * tt + 2,
                                           r0 : r0 + S],
                                        start=(tt == 0),
                                        stop=False,
                                        perf_mode=PM,
                                    )
                            else:
                                for c in range(HC):
                                    nc.tensor.matmul(
                                        dst,
                                        sentT[:, c, r0 : r0 + PADC],
                                        yt[:, c, r0 : r0 + S],
                                        start=(c == 0),
                                        stop=False,
                                    )
                            nc.tensor.matmul(
                                dst,
                                onesr_sb[:],
                                off_sb[0:1,
                                       g * GROWS + r0 : g * GROWS + r0 + S],
                                start=False,
                                stop=True,
                            )
                        qT4 = small.tile([S, 4 * S], fp16, tag="qT",
                                         name=f"qT{g}_{blk}")
                        nc.scalar.activation(
                            qT4[:].rearrange("p (f w) -> p f w", w=S),
                            psb[:S, :].rearrange("p (f w) -> p f w", w=128)[
                                :, :, 0:S
                            ],
                            AF.Sigmoid,
                            bias=bvec_sb[:, 0:1],
                            scale=1.0 / WS,
                        )
                        nc.vector.reduce_sum(
                            out=svg[:, blk * 4 : blk * 4 + 4],
                            in_=qT4[:].rearrange("p (f w) -> p f w", w=S),
                            axis=X,
                        )
                        nc.vector.reciprocal(
                            rg[:, blk * 4 : blk * 4 + 4],
                            svg[:, blk * 4 : blk * 4 + 4],
                        )
                        psb2 = psb2_p.tile([128, 512], fp32, tag="tr",
                                           name=f"tr{g}_{blk}")
                        for q in range(4):
                            bl = blk * 4 + q
                            nc.vector.tensor_scalar(
                                out=NT_cur[:, bl * S : (bl + 1) * S],
                                in0=qT4[:, q * S : (q + 1) * S],
                                scalar1=rg[:, bl : bl + 1],
                                scalar2=LAMB,
                                op0=OP.mult,
                                op1=OP.mult,
                            )
                            nc.tensor.transpose(
                                psb2[:, q * 128 : q * 128 + 50].bitcast(fp16),
                                NT_cur[:, bl * S : bl * S + PADC],
                                eye_sb[:],
                            )
                        ndst = N_cur[
                            :, blk * 4 * SZ : (blk * 4 + 4) * SZ
                        ].rearrange("p (n w) -> p n w", w=SZ)[:, :, 0:S]
                        nsrc = psb2[:S, :].bitcast(fp16).rearrange(
                            "p (f w) -> p f w", w=256
                        )[:, :, 0:S]
                        alt_copy(ndst, nsrc)
                    return t

                out.extend(blk_t(blk) for blk in range(NBLK))
                return out

            packs = []
            p0 = 0
            while p0 < GB:
                packs.append((p0, min(PACK, GB - p0)))
                p0 += PACK

            def solve_thunks(g):
                out = []

                def lvl_pack(j, p0, np_):
                    def t():
                        N_cur, NT_cur = st["N", g], st["NT", g]
                        N_nxt, NT_nxt = st["Nn", g], st["NTn", g]
                        sq = psb_p.tile([128, 512], fp32, tag="sim",
                                        name=f"sq{g}_{j}_{p0}")
                        sqT = psb2_p.tile([128, 512], fp32, tag="tr",
                                          name=f"sqT{g}_{j}_{p0}")
                        for i in range(np_):
                            b = p0 + i
                            nc.tensor.matmul(
                                sq[:, i * SZ : (i + 1) * SZ],
                                NT_cur[:, b * S : b * S + PADC],
                                N_cur[:, b * SZ : (b + 1) * SZ],
                                start=True,
                                stop=True,
                            )
                            nc.tensor.matmul(
                                sqT[:, i * S : (i + 1) * S],
                                N_cur[:, b * SZ : b * SZ + PADC],
                                NT_cur[:, b * S : (b + 1) * S],
                                start=True,
                                stop=True,
                            )
                        sq3 = sq[:S, 0 : np_ * SZ].rearrange(
                            "p (n w) -> p n w", w=SZ
                        )
                        dst3 = N_nxt[:, p0 * SZ : (p0 + np_) * SZ].rearrange(
                            "p (n w) -> p n w", w=SZ
                        )
                        cur_z = N_cur[:, p0 * SZ : (p0 + np_) * SZ].rearrange(
                            "p (n w) -> p n w", w=SZ
                        )[:, :, S:SZ]
                        alt_copy(dst3[:, :, 0:S], sq3[:, :, 0:S])
                        alt_copy(NT_nxt[:, p0 * S : (p0 + np_) * S],
                                 sqT[:S, 0 : np_ * S])
                        nc.vector.tensor_tensor(
                            out=dst3[:, :, S:SZ],
                            in0=sq3[:, :, S:SZ],
                            in1=cur_z,
                            op=OP.add,
                        )
                    return t

                def lvl_start(j):
                    def t():
                        N_nxt = solve_p.tile([S, NPADZ], fp16, tag="Nall",
                                             name=f"N{j + 1}g{g}")
                        NT_nxt = solve_p.tile([S, NPADW], fp16, tag="NTall",
                                              name=f"NT{j + 1}g{g}")
                        st["Nn", g] = N_nxt
                        st["NTn", g] = NT_nxt
                        nc.gpsimd.memset(N_nxt[:, GB * SZ : NPADZ], 0.0)
                        nc.gpsimd.memset(NT_nxt[:, GB * S : NPADW], 0.0)
                    return t

                def lvl_end(j):
                    def t():
                        st["N", g] = st["Nn", g]
                        st["NT", g] = st["NTn", g]
                    return t

                for j in range(NLEV):
                    out.append(lvl_start(j))
                    for p0, np_ in packs:
                        out.append(lvl_pack(j, p0, np_))
                    out.append(lvl_end(j))

                # level NLEV: z' and (N^(2^(NLEV+1)))^T only
                def lvl2_start():
                    NT_nxt = solve_p.tile([S, NPADW], fp16, tag="NTall",
                                          name=f"NTLg{g}")
                    st["NTn", g] = NT_nxt
                    nc.gpsimd.memset(NT_nxt[:, GB * S : NPADW], 0.0)
                    st["sqz", g] = pssv_p.tile([128, 512], fp32, tag="sv",
                                               name=f"sqz{g}")
                out.append(lvl2_start)

                def lvl2_pack(p0, np_):
                    def t():
                        N_cur, NT_cur = st["N", g], st["NT", g]
                        NT_nxt = st["NTn", g]
                        sqz = st["sqz", g]
                        sqT = psb2_p.tile([128, 512], fp32, tag="tr",
                                          name=f"sqTL{g}_{p0}")
                        for i in range(np_):
                            b = p0 + i
                            nc.tensor.matmul(
                                sqz[:, b : b + 1],
                                NT_cur[:, b * S : b * S + PADC],
                                N_cur[:, b * SZ + S : (b + 1) * SZ],
                                start=True,
                                stop=True,
                            )
                            nc.tensor.matmul(
                                sqT[:, i * S : (i + 1) * S],
                                N_cur[:, b * SZ : b * SZ + PADC],
                                NT_cur[:, b * S : (b + 1) * S],
                                start=True,
                                stop=True,
                            )
                        alt_copy(NT_nxt[:, p0 * S : (p0 + np_) * S],
                                 sqT[:S, 0 : np_ * S])
                    return t

                for p0, np_ in packs:
                    out.append(lvl2_pack(p0, np_))

                def lvl2_end():
                    N_cur = st["N", g]
                    sqz = st["sqz", g]
                    znew = grp_p.tile([S, GB], fp16, tag="zn",
                                      name=f"zn{g}")
                    st["zn", g] = znew
                    zc = N_cur[:, 0 : GB * SZ].rearrange(
                        "p (n w) -> p n w", w=SZ
                    )
                    nc.vector.tensor_tensor(
                        out=znew[:].rearrange("p (n w) -> p n w", w=1),
                        in0=sqz[:S, 0:GB].rearrange("p (n w) -> p n w", w=1),
                        in1=zc[:, :, S:SZ],
                        op=OP.add,
                    )
                    st["NT", g] = st["NTn", g]
                out.append(lvl2_end)

                # matvec rounds: w1 = A z3, w2 = A w1, x = z3 + w1 + w2
                def mv1():
                    NT_cur = st["NT", g]
                    znew = st["zn", g]
                    w1p = pssv_p.tile([128, 512], fp32, tag="sv",
                                      name=f"w1p{g}")
                    for bl in range(GB):
                        nc.tensor.matmul(
                            w1p[:, bl : bl + 1],
                            NT_cur[:, bl * S : bl * S + PADC],
                            znew[:, bl : bl + 1],
                            start=True,
                            stop=True,
                        )
                    w1 = grp_p.tile([S, GB], fp16, tag="w1", name=f"w1{g}")
                    st["w1", g] = w1
                    nc.vector.tensor_copy(w1[:], w1p[:S, 0:GB])
                    xt = grp_p.tile([S, GB], fp32, tag="xt", name=f"xt{g}")
                    st["xt", g] = xt
                    nc.vector.tensor_tensor(
                        out=xt[:], in0=znew[:], in1=w1[:], op=OP.add
                    )
                out.append(mv1)

                def mv2():
                    NT_cur = st["NT", g]
                    w1 = st["w1", g]
                    xt = st["xt", g]
                    w2p = pssv_p.tile([128, 512], fp32, tag="sv",
                                      name=f"w2p{g}")
                    for bl in range(GB):
                        nc.tensor.matmul(
                            w2p[:, bl : bl + 1],
                            NT_cur[:, bl * S : bl * S + PADC],
                            w1[:, bl : bl + 1],
                            start=True,
                            stop=True,
                        )
                    xg = grp_p.tile([S, GB], fp32, tag="xg", name=f"xg{g}")
                    nc.vector.tensor_tensor(
                        out=xg[:], in0=w2p[:S, 0:GB], in1=xt[:], op=OP.add
                    )
                    nc.sync.dma_start(
                        out=out32.ap()[g * GB : (g + 1) * GB, :].rearrange(
                            "b s -> s b"
                        ),
                        in_=xg[:],
                    )
                out.append(mv2)
                return out

            # ---- interleaved emission
            from collections import deque

            pending = deque()
            for g in range(NGRP):
                stream_a = []
                if g == 0:
                    stream_a += load_thunk(0)
                if g + 1 < NGRP:
                    stream_a += load_thunk(g + 1)
                stream_a += mm1_thunks(g)
                stream_a += ph2_thunks(g)
                for a in stream_a:
                    a()
                    if pending:
                        pending.popleft()()
                pending.extend(solve_thunks(g))
            while pending:
                pending.popleft()()

    nc.compile()
    _CACHE[key] = nc
    return nc


def _prep(inputs):
    sent = np.ascontiguousarray(np.asarray(inputs["sent_vec"], dtype=np.float32))
    s_r = sent.reshape(NCORES, ROWS, HC, 128)
    sent8 = np.ascontiguousarray(s_r.transpose(0, 3, 2, 1)).astype(E4NP)
    W_rel = np.asarray(inputs["W_rel"], dtype=np.float32)
    W_cont = np.asarray(inputs["W_cont"], dtype=np.float32).reshape(H)
    sb = sent.reshape(B, S, H)
    d = sb.mean(axis=1)
    v = d @ W_rel.T + W_cont[None, :]
    off = np.matmul(sb, v[:, :, None])[:, :, 0]
    off16 = (off * WS).astype(np.float16).reshape(NCORES, 1, ROWS)
    W_sim = np.asarray(inputs["W_sim"], dtype=np.float32) * WS
    wsim8 = np.ascontiguousarray(
        W_sim.reshape(HC, 128, H).transpose(1, 0, 2)
    ).astype(E4NP)
    bval = float(np.asarray(inputs["b_matrix"]).reshape(-1)[0])
    onesr = np.ones((1, PADC), np.float16)
    eye = np.eye(S, dtype=np.float16)
    bvec = np.full((S, 1), bval, np.float32)
    return [
        {
            "sent8": np.ascontiguousarray(sent8[i]),
            "wsim8": wsim8,
            "off16h": np.ascontiguousarray(off16[i]),
            "onesr16": onesr,
            "eye16": eye,
            "bvec32": bvec,
        }
        for i in range(NCORES)
    ]


def _patch_ldw_opt():
    import os

    if os.environ.get("KERNEL_LDW_OPT", "0") != "1":
        return
    import concourse.bass_utils as bu

    if getattr(bu, "_ldw_patched", False):
        return
    orig = bu.run_command

    def run2(argv, **kw):
        argv = [
            "--enable-ldw-opt=true" if a == "--enable-ldw-opt=false" else a
            for a in argv
        ]
        return orig(argv, **kw)

    bu.run_command = run2
    bu._ldw_patched = True


def _run(in_maps, trace=False, **kw):
    from concourse.bass_utils import run_bass_kernel_spmd

    _patch_ldw_opt()
    nc = _get_nc()
    return run_bass_kernel_spmd(nc, in_maps, list(range(NCORES)), trace=trace, **kw)


def kernel(**inputs):
    in_maps = _prep(inputs)
    res = _run(in_maps)
    out = np.concatenate([r["out32"] for r in res.results], axis=0)
    return np.ascontiguousarray(out, dtype=np.float32)


if __name__ == "__main__":
    _get_nc()
    print("build ok")
